# revision 19
# baseline (speedup 1.0000x reference)
"""Trainium2 Bass kernel for DigitConvolutionalModel.

Model: x[B,784] -> reshape 28x28 -> 3x3 valid conv (weights conv_w) ->
[B,676] -> Linear(676,100)+relu -> Linear(100,10)+relu -> Linear(10,10).

The conv is linear, so it folds into the first Linear: W1f = C @ w1 where
C[784,676] is the conv unfold matrix. The whole model becomes a 3-layer MLP
784 -> 100 -> 10 -> 10 with relu between layers.

Sharding: pure data parallel, batch split across 8 cores (8192 rows each).

Precision: x is cast host-side to fp8 e3m4 — halves HBM traffic vs bf16;
weights stay bf16 (mixed-dtype matmul streams at 1 cycle/row). Measured
end-to-end rel err 0.0142 vs the 2e-2 gate.

PE work per 512-batch supertile t:
  - 6 L1 chunk matmuls (128 features each, 0..767) accumulating into
    PSUM bank(t) rows 0:100.
  - For t=4..13 the bank is initialized (start=True) by a fused matmul
    with block stationary S[126,126] (w2 rows 0:100 -> cols 100:110,
    w3 rows 100:110 -> cols 116:126, w1-tail rows 110:126 -> cols
    0:100) over moving fmov(s)[126,512] (h1(s) rows 0:100, h2(s-6)
    rows 100:110, xtail(s+4) rows 110:126), s=t-4: one pass produces
    the L1-tail seed of bank(t), L2(s) at rows 100:110 and L3(s-6) =
    y(s-6) at rows 116:126. Supertiles 0..3 are seeded by standalone
    w1-tail matmuls; banks 14,15 by a w1t-only slice of S (fL1).
  - h1(t) = relu(bank rows 0:100 + b1) -> fmov slot t (ACT).
  - h2(s), s<=9: DVE relu of bank(s+4) rows 100:110 from base 96
    (garbage lanes 96:100 are later overwritten by h1-ACT(s+6)).
  - y(0..3): per-supertile ACT of bank rows 116:126 in idle windows.

Drain: everything else is arranged so the end of the kernel is one
short chain. L2 for s=10..15 comes from six w2-only passes (stationary
S cols 100:110) packing four 10-row outputs per scratch PSUM bank at
partition bands {0,32,64,96} (legal engine AP bases, and DVE lanes are
partition-locked so h2 lands at the same band of its fmov slot);
band-placed stationary SYB replicates w3 at those bands. The final
y's accumulate PARTITION-STACKED: passes with a shifted-column-slice
stationary put each supertile's y at rows 10j..10j+10 of one psum
bank: ybA = y(6..13), ybB = y(4,5,14,15). Stacked passes read only
narrow moving slices ([96:110) for main-style h2 slots, [0:band+10)
for drain slots), so none of them depend on the last h1 ACTs. One
wide ACT / DVE op (+b3 replicated per 10-row group) and one store per
bank replace twelve narrow per-supertile ops, and all of group A plus
y(4,5,15) complete during supertile 14's chunks. The only post-chunk
chain is h1(14) -> L2(14) -> h2(14) -> y(14) pass -> ybB op -> store.

Opening: warmup matmuls on garbage keep the PE busy from the first
post-preamble slot (~7.3us) until real operands land, so the HAM
activity monitor unthrottles the PE clock to 2.4GHz early; the seeds'
operands (w1t + x tails) ride one early gpsimd DMA while the sync
queue delivers bias+chunk0/1 weights, supertile-0 x, then the rest.
"""

import numpy as np
import ml_dtypes

import concourse.bacc as bacc
import concourse.tile as tile
from concourse.tile import add_dep_helper
from concourse import mybir
from concourse.bass_utils import run_bass_kernel_spmd

N_CORES = 8
B = 65536
BC = B // N_CORES  # 8192 rows per core
TN = 512           # batch columns per supertile
NT = BC // TN      # 16 supertiles per core
NKC = 6            # full 128-feature chunks (0..767)
KT = 16            # tail features (768..783)
NF = 784
H1 = 100
HO = 10
F32 = mybir.dt.float32
BF16 = mybir.dt.bfloat16
F8E3 = mybir.dt.float8e3
NP_BF16 = ml_dtypes.bfloat16
NP_F8E3 = ml_dtypes.float8_e3m4

# packed weight blob column layout (bf16 columns)
_C_B = 0                        # [126, 2]  b1/b2/b3 f32 byte-pairs by row
_C_B3R = 2                      # [100, 2]  b3 replicated per 10-row group
_C_B2Q = 4                      # [106, 2]  b2 at bands 0/32/64/96
_C_W1M = 6                      # [128, 600] w1m chunks
_C_FS = 606                     # [126, 126] fused stationary S
_C_SY = 732                     # [110, 190] SYW: w3 at rows 100:110, cols 90:100
_C_SYB = 922                    # [106, 190] SYB: w3 at rows b:b+10 (4 bands)
WBW = 1112

NPAIR = NT // 2
# drain h2 partition band per supertile s=10..15
_BAND = {10: 0, 11: 32, 12: 64, 13: 96, 14: 0, 15: 32}


def _build_nc():
    nc = bacc.Bacc(None, target_bir_lowering=False)

    xt_main = nc.dram_tensor(
        "xt_main", [NT, 128, NKC, TN], F8E3, kind="ExternalInput"
    )
    # tails arranged by fmov slot s (holding xtail(s+4); zeros for s>=12),
    # bf16 so they ride the bf16 fmov tile. Slots 0..15 only: slots 16+
    # are never streamed at rows >=110.
    xt_tail = nc.dram_tensor("xt_tail", [KT, 16, TN], BF16, kind="ExternalInput")
    # tails 0..3 for the standalone bank-seed matmuls + w1t columns
    xtw = nc.dram_tensor("xtw", [KT, 4 * TN + H1], BF16, kind="ExternalInput")
    wblob = nc.dram_tensor("wblob", [128, WBW], BF16, kind="ExternalInput")
    yt = nc.dram_tensor("yt", [HO, 4 * TN], F32, kind="ExternalOutput")
    yt2 = nc.dram_tensor("yt2", [80, TN], F32, kind="ExternalOutput")
    yt3 = nc.dram_tensor("yt3", [40, TN], F32, kind="ExternalOutput")

    relu = mybir.ActivationFunctionType.Relu
    ident = mybir.ActivationFunctionType.Identity

    with tile.TileContext(nc) as tc:
        with (
            tc.tile_pool(name="const", bufs=1) as cpool,
            tc.tile_pool(name="xm", bufs=6) as xpool,
            tc.tile_pool(name="fm", bufs=6) as fpool,
            tc.tile_pool(name="ot", bufs=4) as opool,
            tc.tile_pool(name="psA", bufs=6, space="PSUM") as psA,
            tc.tile_pool(name="psY", bufs=2, space="PSUM") as psY,
        ):
            # seeds' operands on gpsimd (its queue is free; the sync
            # queue starts with what the first chunk matmuls need)
            xtwt = cpool.tile([KT, 4 * TN + H1], BF16, tag="xtw")
            nc.gpsimd.dma_start(xtwt[:], xtw[:])

            wb_s = cpool.tile([128, WBW], BF16, tag="wb")
            xm0 = xpool.tile([128, NKC, TN], F8E3, tag="xm")
            nc.sync.dma_start(wb_s[:, 0:_C_W1M + 2 * H1],
                              wblob[:, 0:_C_W1M + 2 * H1])
            nc.sync.dma_start(xm0[:, 0:2, :], xt_main[0, :, 0:2, :])
            nc.sync.dma_start(wb_s[:, _C_W1M + 2 * H1:_C_FS],
                              wblob[:, _C_W1M + 2 * H1:_C_FS])
            nc.sync.dma_start(xm0[:, 2:6, :], xt_main[0, :, 2:6, :])
            # S/SYW/SYB block after supertile-1's x below: first needed
            # at pair 2 (fused(0)), long after supertile 1's chunks

            # fused-moving tiles pooled PER PAIR (dep tracking intersects
            # partition ranges coarsely — one persistent tile would make
            # every fused matmul wait on the latest h1/h2 write to ANY
            # slot). ptile q covers fmov slots 2q / 2q+1.
            ptiles: dict[int, object] = {}

            def alloc_ptile(q):
                pt = fpool.tile([126, 2, TN], BF16, tag="fm",
                                name=f"ptile{q}")
                ptiles[q] = pt
                if q <= 7:
                    # tails for fmov slots 2q/2q+1 (zeros for slots >= 12)
                    nc.gpsimd.dma_start(pt[110:126, :, :],
                                        xt_tail[:, 2 * q:2 * q + 2, :])
                else:
                    # slots 16..21 are streamed only at rows [0:band+10):
                    # memset below the h2 bands (h2 written by drain STTs)
                    nc.vector.memset(pt[0:96, :, :], 0.0)
                if q <= 2:
                    # no h2(-6..-1) exist for fused(0..5)
                    nc.vector.memset(pt[96:110, :, :], 0.0)
                return pt

            def fmov(s):
                return ptiles[s // 2][:, s % 2, :]

            fs_ap = wb_s[0:126, _C_FS:_C_FS + 126]
            fl1_ap = wb_s[0:126, _C_FS:_C_FS + H1]          # w1t rows only
            l2_ap = wb_s[0:126, _C_FS + 100:_C_FS + 110]    # w2 cols only
            w1t_ap = xtwt[0:KT, 4 * TN:4 * TN + H1]
            b1_ap = wb_s[0:H1, _C_B:_C_B + 2].bitcast(F32)
            bq2_ap = wb_s[96:110, _C_B:_C_B + 2].bitcast(F32)
            bq_ap = wb_s[96:126, _C_B:_C_B + 2].bitcast(F32)
            b3rA_ap = wb_s[0:80, _C_B3R:_C_B3R + 2].bitcast(F32)
            b3rB_ap = wb_s[0:40, _C_B3R:_C_B3R + 2].bitcast(F32)

            prev_mm = [None]

            def mm(out_ap, lhsT_ap, rhs_ap, start, stop, tile_position=None):
                # base-partition 96 APs need an explicit tile_position
                # (auto-derive only accepts 0/32/64)
                m = nc.tensor.matmul(out_ap, lhsT_ap, rhs_ap,
                                     start=start, stop=stop,
                                     skip_group_check=True,
                                     tile_position=tile_position)
                if prev_mm[0] is not None:
                    add_dep_helper(m.ins, prev_mm[0], sync=False,
                                   reason="pe program order")
                prev_mm[0] = m.ins
                return m

            # Warmup on garbage (WAR on purpose: the memset below only
            # exists to satisfy tile allocation and runs afterwards).
            wsc = cpool.tile([128, TN], BF16, tag="wsc")
            wp0 = psA.tile([126, TN], F32, tag="pa")
            wp1 = psA.tile([126, TN], F32, tag="pa")
            wfirst = nc.tensor.matmul(wp0[:], wsc[:, 0:126], wsc[:],
                                      start=True, stop=True)
            prev_mm[0] = wfirst.ins
            for i in range(1, 5):
                w_mm = nc.tensor.matmul((wp1 if i % 2 else wp0)[:],
                                        wsc[:, 0:126], wsc[:],
                                        start=True, stop=True)
                add_dep_helper(w_mm.ins, wfirst.ins, sync=False,
                               reason="warmup weight reuse")
                prev_mm[0] = w_mm.ins
            nc.vector.memset(wsc[:], 0.0)

            banks: dict[int, object] = {}
            ots: dict[int, object] = {}

            alloc_ptile(0)
            alloc_ptile(1)

            # Banks 0..3: chunk 0 opens the accumulation (start=True) and
            # the w1-tail seed closes it AFTER the chunks — its operands
            # (xtw gpsimd DMA) land later than supertile-0's x, so the
            # seed must not gate the first chunk matmuls.
            for t in range(4):
                banks[t] = psA.tile([126, TN], F32, tag="pa", name=f"bank{t}")

            def emit_fused(s):
                """fused(s): L2(s) + L3(s-6) + L1-tail(s+4) -> bank(s+4),
                for s=0..9."""
                bk = psA.tile([126, TN], F32, tag="pa", name=f"bank{s+4}")
                banks[s + 4] = bk
                mm(bk[:], fs_ap, fmov(s), start=True, stop=False)

            def emit_h2(s):
                """h2(s) = relu(bank(s+4)[100:110] + b2) -> fmov(s+6)
                rows 100:110, via DVE STT from base 96 (rows 96:100 are
                garbage lanes, later overwritten by h1-ACT(s+6) and
                zero-weighted in every stationary that reads them)."""
                bk = banks[s + 4]
                nc.vector.scalar_tensor_tensor(
                    ptiles[(s + 6) // 2][96:110, s % 2, :],
                    bk[96:110, :], bq2_ap, wsc[96:110, :],
                    op0=mybir.AluOpType.add, op1=mybir.AluOpType.max)

            def emit_h2_band(s, dscr):
                """drain h2(s) at partition band b=_BAND[s]: relu of
                dscr[b:b+10] + b2 -> fmov(s+6) rows b:b+10 (DVE lanes
                are partition-locked, so in = out partitions)."""
                b = _BAND[s]
                b2q = wb_s[b:b + 10, _C_B2Q:_C_B2Q + 2].bitcast(F32)
                nc.vector.scalar_tensor_tensor(
                    ptiles[(s + 6) // 2][b:b + 10, s % 2, :],
                    dscr[b:b + 10, :], b2q, wsc[b:b + 10, :],
                    op0=mybir.AluOpType.add, op1=mybir.AluOpType.max)

            def emit_y_early(s):
                """y(s-6) for s=6..9: bank(s+4) rows 116:126 + b3 ->
                store. Runs in ACT's idle mid-loop windows."""
                bk = banks[s + 4]
                ot = opool.tile([126, TN], F32, tag="ot", name=f"ot{s-6}")
                nc.scalar.activation(ot[96:126, :], bk[96:126, :],
                                     ident, bias=bq_ap)
                nc.gpsimd.dma_start(
                    yt[:, (s - 6) * TN:(s - 5) * TN], ot[116:126, :]
                )
                ots[s - 6] = ot

            def stacked_main(s, out_ap, ncols, col0, start, stop):
                """stacked L3 pass for a main-style h2 slot (h2(s) at
                fmov(s+6) rows 100:110): reads only [96:110) so it
                never depends on later h1 writes to rows 0:96."""
                c = _C_SY + 90 - col0
                mm(out_ap, wb_s[96:110, c:c + ncols],
                   fmov(s + 6)[96:110, :], start=start, stop=stop,
                   tile_position=(96, 0))

            def stacked_band(s, out_ap, ncols, col0, start, stop):
                """stacked L3 pass for a drain h2 slot (h2(s) at band
                b of fmov(s+6)): reads [0:b+10); lower bands are
                memset zeros."""
                b = _BAND[s]
                c = _C_SYB + 90 - col0
                mm(out_ap, wb_s[0:b + 10, c:c + ncols],
                   fmov(s + 6)[0:b + 10, :], start=start, stop=stop)

            for p in range(NPAIR):
                t0, t1 = 2 * p, 2 * p + 1
                alloc_ptile(p + 2)
                if p == 6:
                    # drain ptile, early so its memset is long done
                    alloc_ptile(10)
                fm = ptiles[p]
                last = p == NPAIR - 1
                # fused passes for the pair-before-last: every dependency
                # (h1 of pair p-2, h2 writes from pair p-1) is at least a
                # full pair old, so the PE never waits here.
                if 2 <= p < 7:
                    emit_fused(2 * p - 4)
                    emit_fused(2 * p - 3)
                if last:
                    # Everything not gated on this pair's chunks runs
                    # now: bank seeds for 14/15 (w1t-only slice of S),
                    # w2-only L2 passes for s=10..13 packed at bands of
                    # one scratch bank, their h2 relus, stacked group-A
                    # L3 passes j=0..3 and group-B y(4),y(5).
                    banks[14] = psA.tile([126, TN], F32, tag="pa",
                                         name="bank14")
                    banks[15] = psA.tile([126, TN], F32, tag="pa",
                                         name="bank15")
                    mm(banks[14][0:H1, :], fl1_ap, fmov(10),
                       start=True, stop=False)
                    mm(banks[15][0:H1, :], fl1_ap, fmov(11),
                       start=True, stop=False)
                    dscrA = psA.tile([126, TN], F32, tag="pa", name="dscrA")
                    # each band pass is its own group: start=True only
                    # clears has_written for the addresses it writes, so
                    # start=False into a fresh band would accumulate onto
                    # stale PSUM from the recycled bank
                    for s in (10, 11, 12, 13):
                        b = _BAND[s]
                        mm(dscrA[b:b + 10, :], l2_ap, fmov(s),
                           start=True, stop=True,
                           tile_position=(0, b))
                    for s in (10, 11, 12, 13):
                        emit_h2_band(s, dscrA)
                    ybA = psY.tile([126, TN], F32, tag="py", name="ybA")
                    ybB = psY.tile([126, TN], F32, tag="py", name="ybB")
                    for j in range(4):  # y(6..9)
                        stacked_main(6 + j, ybA[0:H1, :], H1, 10 * j,
                                     start=(j == 0), stop=False)
                    stacked_main(4, ybB[0:40, :], 40, 0,
                                 start=True, stop=False)   # y(4)
                    stacked_main(5, ybB[0:40, :], 40, 10,
                                 start=False, stop=False)  # y(5)

                if p == 0:
                    xmA = xm0  # DMAs already issued up top, split
                    xmB = xpool.tile([128, NKC, TN], F8E3, tag="xm")
                    nc.sync.dma_start(xmB[:], xt_main[t1])
                    nc.sync.dma_start(wb_s[:, _C_FS:], wblob[:, _C_FS:])
                    order = [(t0, xmA), (t1, xmB)]
                else:
                    xmA = xpool.tile([128, NKC, TN], F8E3, tag="xm")
                    xmB = xpool.tile([128, NKC, TN], F8E3, tag="xm")
                    if last:
                        # supertile 15 first so its h1/L2/h2/y chain
                        # completes during supertile 14's chunks
                        nc.sync.dma_start(xmB[:], xt_main[t1])
                        nc.sync.dma_start(xmA[:], xt_main[t0])
                        order = [(t1, xmB), (t0, xmA)]
                    else:
                        nc.sync.dma_start(xmA[:], xt_main[t0])
                        nc.sync.dma_start(xmB[:], xt_main[t1])
                        order = [(t0, xmA), (t1, xmB)]

                for t, xm in order:
                    for k in range(NKC):
                        mm(banks[t][0:H1, :],
                           wb_s[:, _C_W1M + k * H1:_C_W1M + (k + 1) * H1],
                           xm[:, k, :], start=(t < 4 and k == 0),
                           stop=(t >= 4 and k == NKC - 1))
                    if t < 4:
                        mm(banks[t][0:H1, :], w1t_ap,
                           xtwt[:, t * TN:(t + 1) * TN],
                           start=False, stop=True)
                    nc.scalar.activation(fm[0:H1, t % 2, :],
                                         banks[t][0:H1, :],
                                         relu, bias=b1_ap)
                    if last and t == t1:
                        # stacked y(10..13) first — their h2 STTs
                        # completed during the chunks, so they fill the
                        # PE's h1(15)-ACT shadow — then the 15-chain;
                        # all of this runs during supertile 14's chunks
                        for j in range(4, 8):  # y(10..13)
                            stacked_band(6 + j, ybA[0:H1, :], H1, 10 * j,
                                         start=False, stop=(j == 7))
                        dscrB = psA.tile([126, TN], F32, tag="pa",
                                         name="dscrB")
                        mm(dscrB[32:42, :], l2_ap, fmov(15),
                           start=True, stop=True)
                        emit_h2_band(15, dscrB)

                # deferred bank readers for this pair's fused outputs
                if 2 <= p < 7:
                    s0, s1 = 2 * p - 4, 2 * p - 3
                    emit_h2(s0)
                    emit_h2(s1)
                    if s0 >= 6:
                        emit_y_early(s0)
                        emit_y_early(s1)
                if not last:
                    del banks[t0], banks[t1]

            # ---- drain: the only post-chunk chain ----
            # y(15) pass (its h2 finished during supertile 14's chunks),
            # group-A ACT + store, then h1(14) -> L2(14) -> h2(14) ->
            # y(14) pass -> ybB DVE op -> store.
            stacked_band(15, ybB[0:40, :], 40, 30, start=False, stop=False)
            otA = opool.tile([126, TN], F32, tag="ot", name="otA")
            nc.scalar.activation(otA[0:80, :], ybA[0:80, :],
                                 ident, bias=b3rA_ap)
            nc.gpsimd.dma_start(yt2[:], otA[0:80, :])

            mm(dscrB[0:10, :], l2_ap, fmov(14), start=True, stop=True)
            emit_h2_band(14, dscrB)
            stacked_band(14, ybB[0:40, :], 40, 20, start=False, stop=True)
            otB = opool.tile([126, TN], F32, tag="ot", name="otB")
            nc.vector.scalar_tensor_tensor(
                otB[0:40, :], ybB[0:40, :], b3rB_ap, wsc[0:40, :],
                op0=mybir.AluOpType.add, op1=mybir.AluOpType.add)
            nc.sync.dma_start(yt3[:], otB[0:40, :])

    nc.compile()
    return nc


def _fold_conv_into_w1(conv_w: np.ndarray, w1: np.ndarray) -> np.ndarray:
    """W1f[784,100] such that x @ W1f == conv(x).reshape(B,676) @ w1."""
    c = np.zeros((NF, 26 * 26), dtype=np.float64)
    for di in range(3):
        for dj in range(3):
            ii, jj = np.meshgrid(np.arange(26), np.arange(26), indexing="ij")
            src = (ii + di) * 28 + (jj + dj)
            dst = ii * 26 + jj
            c[src.ravel(), dst.ravel()] += np.float64(conv_w[di, dj])
    return (c @ w1.astype(np.float64)).astype(np.float32)


def _prep_in_maps(x, conv_w, w1, b1, w2, b2, w3, b3):
    x = np.asarray(x, dtype=np.float32)
    conv_w = np.asarray(conv_w, dtype=np.float32)
    w1 = np.asarray(w1, dtype=np.float32)
    b1 = np.asarray(b1, dtype=np.float32)
    w2 = np.asarray(w2, dtype=np.float32)
    b2 = np.asarray(b2, dtype=np.float32)
    w3 = np.asarray(w3, dtype=np.float32)
    b3 = np.asarray(b3, dtype=np.float32)

    w1f = _fold_conv_into_w1(conv_w, w1)  # [784, 100]
    # main chunks: feature f = k*128 + p -> [128, 600]
    w1m = np.ascontiguousarray(
        w1f[: 128 * NKC].reshape(NKC, 128, H1).transpose(1, 0, 2)
    ).astype(NP_BF16).reshape(128, NKC * H1)
    w1t = w1f[128 * NKC:].astype(NP_BF16)  # [16, 100]

    blob = np.zeros((128, WBW), np.uint16)
    bias_rows = np.zeros((126, 1), np.float32)
    bias_rows[0:H1, 0] = b1
    bias_rows[100:110, 0] = b2
    bias_rows[116:126, 0] = b3
    blob[0:126, _C_B:_C_B + 2] = bias_rows.view(np.uint16)
    b3rep = np.tile(b3, 10).reshape(100, 1).astype(np.float32)
    blob[0:100, _C_B3R:_C_B3R + 2] = b3rep.view(np.uint16)
    b2q = np.zeros((106, 1), np.float32)
    for b in (0, 32, 64, 96):
        b2q[b:b + 10, 0] = b2
    blob[0:106, _C_B2Q:_C_B2Q + 2] = b2q.view(np.uint16)
    blob[:, _C_W1M:_C_W1M + NKC * H1] = w1m.view(np.uint16)
    # fused stationary S[126,126]
    s_blk = np.zeros((126, 126), np.float32)
    s_blk[0:H1, 100:110] = w2
    s_blk[100:110, 116:126] = w3
    s_blk[110:126, 0:H1] = w1t.astype(np.float32)
    blob[0:126, _C_FS:_C_FS + 126] = s_blk.astype(NP_BF16).view(np.uint16)
    # stacked-L3 stationaries: SYW (w3 at rows 100:110, cols 90:100),
    # SYB (w3 at rows b:b+10 for bands b in {0,32,64,96}, cols 90:100)
    syw = np.zeros((110, 190), np.float32)
    syw[100:110, 90:100] = w3
    blob[0:110, _C_SY:_C_SY + 190] = syw.astype(NP_BF16).view(np.uint16)
    syb = np.zeros((106, 190), np.float32)
    for b in (0, 32, 64, 96):
        syb[b:b + 10, 90:100] = w3
    blob[0:106, _C_SYB:_C_SYB + 190] = syb.astype(NP_BF16).view(np.uint16)
    shared = {"wblob": blob.view(NP_BF16)}

    xb = x.astype(NP_F8E3)  # cast once, full batch
    in_maps = []
    for core in range(N_CORES):
        xc = xb[core * BC:(core + 1) * BC]  # [8192, 784] f8e3
        xct = xc.reshape(NT, TN, NF).transpose(0, 2, 1)  # [NT, NF, TN]
        xt_main = np.ascontiguousarray(
            xct[:, : 128 * NKC].reshape(NT, NKC, 128, TN).transpose(0, 2, 1, 3)
        )  # [NT, 128, NKC, TN]
        tails = xct[:, 128 * NKC:].astype(NP_BF16)  # [NT, KT, TN]
        # fmov slot s holds xtail(s+4); slots 12..15 stay zero
        xt_tail = np.zeros((KT, 16, TN), NP_BF16)
        xt_tail[:, 0:12, :] = tails[4:16].transpose(1, 0, 2)
        xtw = np.zeros((KT, 4 * TN + H1), NP_BF16)
        xtw[:, 0:4 * TN] = tails[0:4].transpose(1, 0, 2).reshape(KT, 4 * TN)
        xtw[:, 4 * TN:] = w1t
        in_maps.append({"xt_main": xt_main, "xt_tail": xt_tail,
                        "xtw": xtw, **shared})
    return in_maps


_NC = None


def _get_nc():
    global _NC
    if _NC is None:
        _NC = _build_nc()
    return _NC


def _assemble(results):
    out = np.empty((B, HO), dtype=np.float32)
    for i in range(N_CORES):
        o = out[i * BC:(i + 1) * BC]
        r = results[i]
        # y(0..3) from yt, y(4,5,14,15) from yt3, y(6..13) from yt2
        o[0:4 * TN] = r["yt"].T
        y3 = r["yt3"].reshape(4, HO, TN)
        o[4 * TN:5 * TN] = y3[0].T
        o[5 * TN:6 * TN] = y3[1].T
        o[6 * TN:14 * TN] = r["yt2"].reshape(8, HO, TN).transpose(
            0, 2, 1).reshape(8 * TN, HO)
        o[14 * TN:15 * TN] = y3[2].T
        o[15 * TN:16 * TN] = y3[3].T
    return out


def kernel(x, conv_w, w1, b1, w2, b2, w3, b3):
    in_maps = _prep_in_maps(x, conv_w, w1, b1, w2, b2, w3, b3)
    nc = _get_nc()
    res = run_bass_kernel_spmd(nc, in_maps, core_ids=list(range(N_CORES)))
    return _assemble(res.results)


if __name__ == "__main__":
    rng = np.random.default_rng(0)
    inputs = {
        "x": rng.standard_normal((B, NF), dtype=np.float32),
        "conv_w": np.ones((3, 3), dtype=np.float32),
        "w1": (rng.standard_normal((676, H1)) * 0.04).astype(np.float32),
        "b1": np.zeros(H1, dtype=np.float32),
        "w2": (rng.standard_normal((H1, HO)) * 0.1).astype(np.float32),
        "b2": np.zeros(HO, dtype=np.float32),
        "w3": (rng.standard_normal((HO, HO)) * 0.3).astype(np.float32),
        "b3": np.zeros(HO, dtype=np.float32),
    }
    out = kernel(**inputs)
    print(out.shape, out.dtype)


# revision 20
# speedup vs baseline: 1.0019x; 1.0019x over previous
"""Trainium2 Bass kernel for DigitConvolutionalModel.

Model: x[B,784] -> reshape 28x28 -> 3x3 valid conv (weights conv_w) ->
[B,676] -> Linear(676,100)+relu -> Linear(100,10)+relu -> Linear(10,10).

The conv is linear, so it folds into the first Linear: W1f = C @ w1 where
C[784,676] is the conv unfold matrix. The whole model becomes a 3-layer MLP
784 -> 100 -> 10 -> 10 with relu between layers.

Sharding: pure data parallel, batch split across 8 cores (8192 rows each).

Precision: x is cast host-side to fp8 e3m4 — halves HBM traffic vs bf16;
weights stay bf16 (mixed-dtype matmul streams at 1 cycle/row). Measured
end-to-end rel err 0.0142 vs the 2e-2 gate.

PE work per 512-batch supertile t:
  - 6 L1 chunk matmuls (128 features each, 0..767) accumulating into
    PSUM bank(t) rows 0:100.
  - For t=4..13 the bank is initialized (start=True) by a fused matmul
    with block stationary S[126,126] (w2 rows 0:100 -> cols 100:110,
    w3 rows 100:110 -> cols 116:126, w1-tail rows 110:126 -> cols
    0:100) over moving fmov(s)[126,512] (h1(s) rows 0:100, h2(s-6)
    rows 100:110, xtail(s+4) rows 110:126), s=t-4: one pass produces
    the L1-tail seed of bank(t), L2(s) at rows 100:110 and L3(s-6) =
    y(s-6) at rows 116:126. Supertiles 0..3 are seeded by standalone
    w1-tail matmuls; banks 14,15 by a w1t-only slice of S (fL1).
  - h1(t) = relu(bank rows 0:100 + b1) -> fmov slot t (ACT).
  - h2(s), s<=9: DVE relu of bank(s+4) rows 100:110 from base 96
    (garbage lanes 96:100 are later overwritten by h1-ACT(s+6)).
  - y(0..3): per-supertile ACT of bank rows 116:126 in idle windows.

Drain: everything else is arranged so the end of the kernel is one
short chain. L2 for s=10..15 comes from six w2-only passes (stationary
S cols 100:110) packing four 10-row outputs per scratch PSUM bank at
partition bands {0,32,64,96} (legal engine AP bases, and DVE lanes are
partition-locked so h2 lands at the same band of its fmov slot);
band-placed stationary SYB replicates w3 at those bands. The final
y's accumulate PARTITION-STACKED: passes with a shifted-column-slice
stationary put each supertile's y at rows 10j..10j+10 of one psum
bank: ybA = y(6..13), ybB = y(4,5,14,15). Stacked passes read only
narrow moving slices ([96:110) for main-style h2 slots, [0:band+10)
for drain slots), so none of them depend on the last h1 ACTs. One
wide ACT / DVE op (+b3 replicated per 10-row group) and one store per
bank replace twelve narrow per-supertile ops, and all of group A plus
y(4,5,15) complete during supertile 14's chunks. The only post-chunk
chain is h1(14) -> L2(14) -> h2(14) -> y(14) pass -> ybB op -> store.

Opening: warmup matmuls on garbage keep the PE busy from the first
post-preamble slot (~7.3us) until real operands land, so the HAM
activity monitor unthrottles the PE clock to 2.4GHz early; the seeds'
operands (w1t + x tails) ride one early gpsimd DMA while the sync
queue delivers bias+chunk0/1 weights, supertile-0 x, then the rest.
"""

import numpy as np
import ml_dtypes

import concourse.bacc as bacc
import concourse.tile as tile
from concourse.tile import add_dep_helper
from concourse import mybir
from concourse.bass_utils import run_bass_kernel_spmd

N_CORES = 8
B = 65536
BC = B // N_CORES  # 8192 rows per core
TN = 512           # batch columns per supertile
NT = BC // TN      # 16 supertiles per core
NKC = 6            # full 128-feature chunks (0..767)
KT = 16            # tail features (768..783)
NF = 784
H1 = 100
HO = 10
F32 = mybir.dt.float32
BF16 = mybir.dt.bfloat16
F8E3 = mybir.dt.float8e3
NP_BF16 = ml_dtypes.bfloat16
NP_F8E3 = ml_dtypes.float8_e3m4

# packed weight blob column layout (bf16 columns)
_C_B = 0                        # [126, 2]  b1/b2/b3 f32 byte-pairs by row
_C_B3R = 2                      # [100, 2]  b3 replicated per 10-row group
_C_B2Q = 4                      # [106, 2]  b2 at bands 0/32/64/96
_C_W1M = 6                      # [128, 600] w1m chunks
_C_FS = 606                     # [126, 126] fused stationary S
_C_SY = 732                     # [110, 190] SYW: w3 at rows 100:110, cols 90:100
_C_SYB = 922                    # [106, 190] SYB: w3 at rows b:b+10 (4 bands)
WBW = 1112

NPAIR = NT // 2
# drain h2 partition band per supertile s=10..15
_BAND = {10: 0, 11: 32, 12: 64, 13: 96, 14: 0, 15: 32}


def _build_nc():
    nc = bacc.Bacc(None, target_bir_lowering=False)

    xt_main = nc.dram_tensor(
        "xt_main", [NT, 128, NKC, TN], F8E3, kind="ExternalInput"
    )
    # tails arranged by fmov slot s (holding xtail(s+4); zeros for s>=12),
    # bf16 so they ride the bf16 fmov tile. Slots 0..15 only: slots 16+
    # are never streamed at rows >=110.
    xt_tail = nc.dram_tensor("xt_tail", [KT, 16, TN], BF16, kind="ExternalInput")
    # tails 0..3 for the standalone bank-seed matmuls + w1t columns
    xtw = nc.dram_tensor("xtw", [KT, 4 * TN + H1], BF16, kind="ExternalInput")
    wblob = nc.dram_tensor("wblob", [128, WBW], BF16, kind="ExternalInput")
    yt = nc.dram_tensor("yt", [HO, 4 * TN], F32, kind="ExternalOutput")
    yt2 = nc.dram_tensor("yt2", [80, TN], F32, kind="ExternalOutput")
    yt3 = nc.dram_tensor("yt3", [40, TN], F32, kind="ExternalOutput")

    relu = mybir.ActivationFunctionType.Relu
    ident = mybir.ActivationFunctionType.Identity

    with tile.TileContext(nc) as tc:
        with (
            tc.tile_pool(name="const", bufs=1) as cpool,
            tc.tile_pool(name="xm", bufs=6) as xpool,
            tc.tile_pool(name="fm", bufs=6) as fpool,
            tc.tile_pool(name="ot", bufs=4) as opool,
            tc.tile_pool(name="psA", bufs=6, space="PSUM") as psA,
            tc.tile_pool(name="psY", bufs=2, space="PSUM") as psY,
        ):
            # Opening DMAs split across BOTH queues so the two receipt
            # latencies overlap: sync carries bias+chunk0/1 weights,
            # supertile-0 chunks 0:2, then the rest of w1m; gpsimd
            # carries the seeds' operands and supertile-0 chunks 2:6.
            xtwt = cpool.tile([KT, 4 * TN + H1], BF16, tag="xtw")
            nc.gpsimd.dma_start(xtwt[:], xtw[:])

            wb_s = cpool.tile([128, WBW], BF16, tag="wb")
            xm0 = xpool.tile([128, NKC, TN], F8E3, tag="xm")
            nc.sync.dma_start(wb_s[:, 0:_C_W1M + 2 * H1],
                              wblob[:, 0:_C_W1M + 2 * H1])
            nc.sync.dma_start(xm0[:, 0:2, :], xt_main[0, :, 0:2, :])
            nc.gpsimd.dma_start(xm0[:, 2:6, :], xt_main[0, :, 2:6, :])
            nc.sync.dma_start(wb_s[:, _C_W1M + 2 * H1:_C_FS],
                              wblob[:, _C_W1M + 2 * H1:_C_FS])
            # S/SYW/SYB block after supertile-1's x below: first needed
            # at pair 2 (fused(0)), long after supertile 1's chunks

            # fused-moving tiles pooled PER PAIR (dep tracking intersects
            # partition ranges coarsely — one persistent tile would make
            # every fused matmul wait on the latest h1/h2 write to ANY
            # slot). ptile q covers fmov slots 2q / 2q+1.
            ptiles: dict[int, object] = {}

            def alloc_ptile(q):
                pt = fpool.tile([126, 2, TN], BF16, tag="fm",
                                name=f"ptile{q}")
                ptiles[q] = pt
                if q <= 7:
                    # tails for fmov slots 2q/2q+1 (zeros for slots >= 12)
                    nc.gpsimd.dma_start(pt[110:126, :, :],
                                        xt_tail[:, 2 * q:2 * q + 2, :])
                else:
                    # slots 16..21 are streamed only at rows [0:band+10):
                    # memset below the h2 bands (h2 written by drain STTs)
                    nc.vector.memset(pt[0:96, :, :], 0.0)
                if q <= 2:
                    # no h2(-6..-1) exist for fused(0..5)
                    nc.vector.memset(pt[96:110, :, :], 0.0)
                return pt

            def fmov(s):
                return ptiles[s // 2][:, s % 2, :]

            fs_ap = wb_s[0:126, _C_FS:_C_FS + 126]
            fl1_ap = wb_s[0:126, _C_FS:_C_FS + H1]          # w1t rows only
            l2_ap = wb_s[0:126, _C_FS + 100:_C_FS + 110]    # w2 cols only
            w1t_ap = xtwt[0:KT, 4 * TN:4 * TN + H1]
            b1_ap = wb_s[0:H1, _C_B:_C_B + 2].bitcast(F32)
            bq2_ap = wb_s[96:110, _C_B:_C_B + 2].bitcast(F32)
            bq_ap = wb_s[96:126, _C_B:_C_B + 2].bitcast(F32)
            b3rA_ap = wb_s[0:80, _C_B3R:_C_B3R + 2].bitcast(F32)
            b3rB_ap = wb_s[0:40, _C_B3R:_C_B3R + 2].bitcast(F32)

            prev_mm = [None]

            def mm(out_ap, lhsT_ap, rhs_ap, start, stop, tile_position=None):
                # base-partition 96 APs need an explicit tile_position
                # (auto-derive only accepts 0/32/64)
                m = nc.tensor.matmul(out_ap, lhsT_ap, rhs_ap,
                                     start=start, stop=stop,
                                     skip_group_check=True,
                                     tile_position=tile_position)
                if prev_mm[0] is not None:
                    add_dep_helper(m.ins, prev_mm[0], sync=False,
                                   reason="pe program order")
                prev_mm[0] = m.ins
                return m

            # Warmup on garbage (WAR on purpose: the memset below only
            # exists to satisfy tile allocation and runs afterwards).
            wsc = cpool.tile([128, TN], BF16, tag="wsc")
            wp0 = psA.tile([126, TN], F32, tag="pa")
            wp1 = psA.tile([126, TN], F32, tag="pa")
            wfirst = nc.tensor.matmul(wp0[:], wsc[:, 0:126], wsc[:],
                                      start=True, stop=True)
            prev_mm[0] = wfirst.ins
            for i in range(1, 5):
                w_mm = nc.tensor.matmul((wp1 if i % 2 else wp0)[:],
                                        wsc[:, 0:126], wsc[:],
                                        start=True, stop=True)
                add_dep_helper(w_mm.ins, wfirst.ins, sync=False,
                               reason="warmup weight reuse")
                prev_mm[0] = w_mm.ins
            nc.vector.memset(wsc[:], 0.0)

            banks: dict[int, object] = {}
            ots: dict[int, object] = {}

            alloc_ptile(0)
            alloc_ptile(1)

            # Banks 0..3: chunk 0 opens the accumulation (start=True) and
            # the w1-tail seed closes it AFTER the chunks — its operands
            # (xtw gpsimd DMA) land later than supertile-0's x, so the
            # seed must not gate the first chunk matmuls.
            for t in range(4):
                banks[t] = psA.tile([126, TN], F32, tag="pa", name=f"bank{t}")

            def emit_fused(s):
                """fused(s): L2(s) + L3(s-6) + L1-tail(s+4) -> bank(s+4),
                for s=0..9."""
                bk = psA.tile([126, TN], F32, tag="pa", name=f"bank{s+4}")
                banks[s + 4] = bk
                mm(bk[:], fs_ap, fmov(s), start=True, stop=False)

            def emit_h2(s):
                """h2(s) = relu(bank(s+4)[100:110] + b2) -> fmov(s+6)
                rows 100:110, via DVE STT from base 96 (rows 96:100 are
                garbage lanes, later overwritten by h1-ACT(s+6) and
                zero-weighted in every stationary that reads them)."""
                bk = banks[s + 4]
                nc.vector.scalar_tensor_tensor(
                    ptiles[(s + 6) // 2][96:110, s % 2, :],
                    bk[96:110, :], bq2_ap, wsc[96:110, :],
                    op0=mybir.AluOpType.add, op1=mybir.AluOpType.max)

            def emit_h2_band(s, dscr):
                """drain h2(s) at partition band b=_BAND[s]: relu of
                dscr[b:b+10] + b2 -> fmov(s+6) rows b:b+10 (DVE lanes
                are partition-locked, so in = out partitions)."""
                b = _BAND[s]
                b2q = wb_s[b:b + 10, _C_B2Q:_C_B2Q + 2].bitcast(F32)
                nc.vector.scalar_tensor_tensor(
                    ptiles[(s + 6) // 2][b:b + 10, s % 2, :],
                    dscr[b:b + 10, :], b2q, wsc[b:b + 10, :],
                    op0=mybir.AluOpType.add, op1=mybir.AluOpType.max)

            def emit_y_early(s):
                """y(s-6) for s=6..9: bank(s+4) rows 116:126 + b3 ->
                store. Runs in ACT's idle mid-loop windows."""
                bk = banks[s + 4]
                ot = opool.tile([126, TN], F32, tag="ot", name=f"ot{s-6}")
                nc.scalar.activation(ot[96:126, :], bk[96:126, :],
                                     ident, bias=bq_ap)
                nc.gpsimd.dma_start(
                    yt[:, (s - 6) * TN:(s - 5) * TN], ot[116:126, :]
                )
                ots[s - 6] = ot

            def stacked_main(s, out_ap, ncols, col0, start, stop):
                """stacked L3 pass for a main-style h2 slot (h2(s) at
                fmov(s+6) rows 100:110): reads only [96:110) so it
                never depends on later h1 writes to rows 0:96."""
                c = _C_SY + 90 - col0
                mm(out_ap, wb_s[96:110, c:c + ncols],
                   fmov(s + 6)[96:110, :], start=start, stop=stop,
                   tile_position=(96, 0))

            def stacked_band(s, out_ap, ncols, col0, start, stop):
                """stacked L3 pass for a drain h2 slot (h2(s) at band
                b of fmov(s+6)): reads [0:b+10); lower bands are
                memset zeros."""
                b = _BAND[s]
                c = _C_SYB + 90 - col0
                mm(out_ap, wb_s[0:b + 10, c:c + ncols],
                   fmov(s + 6)[0:b + 10, :], start=start, stop=stop)

            for p in range(NPAIR):
                t0, t1 = 2 * p, 2 * p + 1
                alloc_ptile(p + 2)
                if p == 6:
                    # drain ptile, early so its memset is long done
                    alloc_ptile(10)
                fm = ptiles[p]
                last = p == NPAIR - 1
                # fused passes for the pair-before-last: every dependency
                # (h1 of pair p-2, h2 writes from pair p-1) is at least a
                # full pair old, so the PE never waits here.
                if 2 <= p < 7:
                    emit_fused(2 * p - 4)
                    emit_fused(2 * p - 3)
                if last:
                    # Everything not gated on this pair's chunks runs
                    # now: bank seeds for 14/15 (w1t-only slice of S),
                    # w2-only L2 passes for s=10..13 packed at bands of
                    # one scratch bank, their h2 relus, stacked group-A
                    # L3 passes j=0..3 and group-B y(4),y(5).
                    banks[14] = psA.tile([126, TN], F32, tag="pa",
                                         name="bank14")
                    banks[15] = psA.tile([126, TN], F32, tag="pa",
                                         name="bank15")
                    mm(banks[14][0:H1, :], fl1_ap, fmov(10),
                       start=True, stop=False)
                    mm(banks[15][0:H1, :], fl1_ap, fmov(11),
                       start=True, stop=False)
                    dscrA = psA.tile([126, TN], F32, tag="pa", name="dscrA")
                    # each band pass is its own group: start=True only
                    # clears has_written for the addresses it writes, so
                    # start=False into a fresh band would accumulate onto
                    # stale PSUM from the recycled bank
                    for s in (10, 11, 12, 13):
                        b = _BAND[s]
                        mm(dscrA[b:b + 10, :], l2_ap, fmov(s),
                           start=True, stop=True,
                           tile_position=(0, b))
                    for s in (10, 11, 12, 13):
                        emit_h2_band(s, dscrA)
                    ybA = psY.tile([126, TN], F32, tag="py", name="ybA")
                    ybB = psY.tile([126, TN], F32, tag="py", name="ybB")
                    for j in range(4):  # y(6..9)
                        stacked_main(6 + j, ybA[0:H1, :], H1, 10 * j,
                                     start=(j == 0), stop=False)
                    stacked_main(4, ybB[0:40, :], 40, 0,
                                 start=True, stop=False)   # y(4)
                    stacked_main(5, ybB[0:40, :], 40, 10,
                                 start=False, stop=False)  # y(5)

                if p == 0:
                    xmA = xm0  # DMAs already issued up top, split
                    xmB = xpool.tile([128, NKC, TN], F8E3, tag="xm")
                    nc.sync.dma_start(xmB[:], xt_main[t1])
                    nc.sync.dma_start(wb_s[:, _C_FS:], wblob[:, _C_FS:])
                    order = [(t0, xmA), (t1, xmB)]
                else:
                    xmA = xpool.tile([128, NKC, TN], F8E3, tag="xm")
                    xmB = xpool.tile([128, NKC, TN], F8E3, tag="xm")
                    if last:
                        # supertile 15 first so its h1/L2/h2/y chain
                        # completes during supertile 14's chunks
                        nc.sync.dma_start(xmB[:], xt_main[t1])
                        nc.sync.dma_start(xmA[:], xt_main[t0])
                        order = [(t1, xmB), (t0, xmA)]
                    else:
                        nc.sync.dma_start(xmA[:], xt_main[t0])
                        nc.sync.dma_start(xmB[:], xt_main[t1])
                        order = [(t0, xmA), (t1, xmB)]

                for t, xm in order:
                    for k in range(NKC):
                        mm(banks[t][0:H1, :],
                           wb_s[:, _C_W1M + k * H1:_C_W1M + (k + 1) * H1],
                           xm[:, k, :], start=(t < 4 and k == 0),
                           stop=(t >= 4 and k == NKC - 1))
                    if t < 4:
                        mm(banks[t][0:H1, :], w1t_ap,
                           xtwt[:, t * TN:(t + 1) * TN],
                           start=False, stop=True)
                    nc.scalar.activation(fm[0:H1, t % 2, :],
                                         banks[t][0:H1, :],
                                         relu, bias=b1_ap)
                    if last and t == t1:
                        # stacked y(10..13) first — their h2 STTs
                        # completed during the chunks, so they fill the
                        # PE's h1(15)-ACT shadow — then the 15-chain;
                        # all of this runs during supertile 14's chunks
                        for j in range(4, 8):  # y(10..13)
                            stacked_band(6 + j, ybA[0:H1, :], H1, 10 * j,
                                         start=False, stop=(j == 7))
                        dscrB = psA.tile([126, TN], F32, tag="pa",
                                         name="dscrB")
                        mm(dscrB[32:42, :], l2_ap, fmov(15),
                           start=True, stop=True)
                        emit_h2_band(15, dscrB)

                # deferred bank readers for this pair's fused outputs
                if 2 <= p < 7:
                    s0, s1 = 2 * p - 4, 2 * p - 3
                    emit_h2(s0)
                    emit_h2(s1)
                    if s0 >= 6:
                        emit_y_early(s0)
                        emit_y_early(s1)
                if not last:
                    del banks[t0], banks[t1]

            # ---- drain: the only post-chunk chain ----
            # y(15) pass (its h2 finished during supertile 14's chunks),
            # group-A ACT + store, then h1(14) -> L2(14) -> h2(14) ->
            # y(14) pass -> ybB DVE op -> store.
            stacked_band(15, ybB[0:40, :], 40, 30, start=False, stop=False)
            otA = opool.tile([126, TN], F32, tag="ot", name="otA")
            nc.scalar.activation(otA[0:80, :], ybA[0:80, :],
                                 ident, bias=b3rA_ap)
            nc.gpsimd.dma_start(yt2[:], otA[0:80, :])

            mm(dscrB[0:10, :], l2_ap, fmov(14), start=True, stop=True)
            emit_h2_band(14, dscrB)
            stacked_band(14, ybB[0:40, :], 40, 20, start=False, stop=True)
            otB = opool.tile([126, TN], F32, tag="ot", name="otB")
            nc.vector.scalar_tensor_tensor(
                otB[0:40, :], ybB[0:40, :], b3rB_ap, wsc[0:40, :],
                op0=mybir.AluOpType.add, op1=mybir.AluOpType.add)
            nc.sync.dma_start(yt3[:], otB[0:40, :])

    nc.compile()
    return nc


def _fold_conv_into_w1(conv_w: np.ndarray, w1: np.ndarray) -> np.ndarray:
    """W1f[784,100] such that x @ W1f == conv(x).reshape(B,676) @ w1."""
    c = np.zeros((NF, 26 * 26), dtype=np.float64)
    for di in range(3):
        for dj in range(3):
            ii, jj = np.meshgrid(np.arange(26), np.arange(26), indexing="ij")
            src = (ii + di) * 28 + (jj + dj)
            dst = ii * 26 + jj
            c[src.ravel(), dst.ravel()] += np.float64(conv_w[di, dj])
    return (c @ w1.astype(np.float64)).astype(np.float32)


def _prep_in_maps(x, conv_w, w1, b1, w2, b2, w3, b3):
    x = np.asarray(x, dtype=np.float32)
    conv_w = np.asarray(conv_w, dtype=np.float32)
    w1 = np.asarray(w1, dtype=np.float32)
    b1 = np.asarray(b1, dtype=np.float32)
    w2 = np.asarray(w2, dtype=np.float32)
    b2 = np.asarray(b2, dtype=np.float32)
    w3 = np.asarray(w3, dtype=np.float32)
    b3 = np.asarray(b3, dtype=np.float32)

    w1f = _fold_conv_into_w1(conv_w, w1)  # [784, 100]
    # main chunks: feature f = k*128 + p -> [128, 600]
    w1m = np.ascontiguousarray(
        w1f[: 128 * NKC].reshape(NKC, 128, H1).transpose(1, 0, 2)
    ).astype(NP_BF16).reshape(128, NKC * H1)
    w1t = w1f[128 * NKC:].astype(NP_BF16)  # [16, 100]

    blob = np.zeros((128, WBW), np.uint16)
    bias_rows = np.zeros((126, 1), np.float32)
    bias_rows[0:H1, 0] = b1
    bias_rows[100:110, 0] = b2
    bias_rows[116:126, 0] = b3
    blob[0:126, _C_B:_C_B + 2] = bias_rows.view(np.uint16)
    b3rep = np.tile(b3, 10).reshape(100, 1).astype(np.float32)
    blob[0:100, _C_B3R:_C_B3R + 2] = b3rep.view(np.uint16)
    b2q = np.zeros((106, 1), np.float32)
    for b in (0, 32, 64, 96):
        b2q[b:b + 10, 0] = b2
    blob[0:106, _C_B2Q:_C_B2Q + 2] = b2q.view(np.uint16)
    blob[:, _C_W1M:_C_W1M + NKC * H1] = w1m.view(np.uint16)
    # fused stationary S[126,126]
    s_blk = np.zeros((126, 126), np.float32)
    s_blk[0:H1, 100:110] = w2
    s_blk[100:110, 116:126] = w3
    s_blk[110:126, 0:H1] = w1t.astype(np.float32)
    blob[0:126, _C_FS:_C_FS + 126] = s_blk.astype(NP_BF16).view(np.uint16)
    # stacked-L3 stationaries: SYW (w3 at rows 100:110, cols 90:100),
    # SYB (w3 at rows b:b+10 for bands b in {0,32,64,96}, cols 90:100)
    syw = np.zeros((110, 190), np.float32)
    syw[100:110, 90:100] = w3
    blob[0:110, _C_SY:_C_SY + 190] = syw.astype(NP_BF16).view(np.uint16)
    syb = np.zeros((106, 190), np.float32)
    for b in (0, 32, 64, 96):
        syb[b:b + 10, 90:100] = w3
    blob[0:106, _C_SYB:_C_SYB + 190] = syb.astype(NP_BF16).view(np.uint16)
    shared = {"wblob": blob.view(NP_BF16)}

    xb = x.astype(NP_F8E3)  # cast once, full batch
    in_maps = []
    for core in range(N_CORES):
        xc = xb[core * BC:(core + 1) * BC]  # [8192, 784] f8e3
        xct = xc.reshape(NT, TN, NF).transpose(0, 2, 1)  # [NT, NF, TN]
        xt_main = np.ascontiguousarray(
            xct[:, : 128 * NKC].reshape(NT, NKC, 128, TN).transpose(0, 2, 1, 3)
        )  # [NT, 128, NKC, TN]
        tails = xct[:, 128 * NKC:].astype(NP_BF16)  # [NT, KT, TN]
        # fmov slot s holds xtail(s+4); slots 12..15 stay zero
        xt_tail = np.zeros((KT, 16, TN), NP_BF16)
        xt_tail[:, 0:12, :] = tails[4:16].transpose(1, 0, 2)
        xtw = np.zeros((KT, 4 * TN + H1), NP_BF16)
        xtw[:, 0:4 * TN] = tails[0:4].transpose(1, 0, 2).reshape(KT, 4 * TN)
        xtw[:, 4 * TN:] = w1t
        in_maps.append({"xt_main": xt_main, "xt_tail": xt_tail,
                        "xtw": xtw, **shared})
    return in_maps


_NC = None


def _get_nc():
    global _NC
    if _NC is None:
        _NC = _build_nc()
    return _NC


def _assemble(results):
    out = np.empty((B, HO), dtype=np.float32)
    for i in range(N_CORES):
        o = out[i * BC:(i + 1) * BC]
        r = results[i]
        # y(0..3) from yt, y(4,5,14,15) from yt3, y(6..13) from yt2
        o[0:4 * TN] = r["yt"].T
        y3 = r["yt3"].reshape(4, HO, TN)
        o[4 * TN:5 * TN] = y3[0].T
        o[5 * TN:6 * TN] = y3[1].T
        o[6 * TN:14 * TN] = r["yt2"].reshape(8, HO, TN).transpose(
            0, 2, 1).reshape(8 * TN, HO)
        o[14 * TN:15 * TN] = y3[2].T
        o[15 * TN:16 * TN] = y3[3].T
    return out


def kernel(x, conv_w, w1, b1, w2, b2, w3, b3):
    in_maps = _prep_in_maps(x, conv_w, w1, b1, w2, b2, w3, b3)
    nc = _get_nc()
    res = run_bass_kernel_spmd(nc, in_maps, core_ids=list(range(N_CORES)))
    return _assemble(res.results)


if __name__ == "__main__":
    rng = np.random.default_rng(0)
    inputs = {
        "x": rng.standard_normal((B, NF), dtype=np.float32),
        "conv_w": np.ones((3, 3), dtype=np.float32),
        "w1": (rng.standard_normal((676, H1)) * 0.04).astype(np.float32),
        "b1": np.zeros(H1, dtype=np.float32),
        "w2": (rng.standard_normal((H1, HO)) * 0.1).astype(np.float32),
        "b2": np.zeros(HO, dtype=np.float32),
        "w3": (rng.standard_normal((HO, HO)) * 0.3).astype(np.float32),
        "b3": np.zeros(HO, dtype=np.float32),
    }
    out = kernel(**inputs)
    print(out.shape, out.dtype)
